# revision 14
# baseline (speedup 1.0000x reference)
"""Multi-head attention (conv1x1 projections) on 8 Trainium2 NeuronCores.

Reference computation (B=2, L=2048, C=256, H=W=4, 8 heads x d_head=32):
    q = einsum('blchw,oc->blohw', query, Wq)   (same for k, v)
    per flattened batch fb = head*B + b:
      score = q_fb @ k_fb^T / sqrt(32)   -> softmax over keys -> attn
      context = attn @ v_fb
Returns (context [B,L,C,H,W], attn [16,L,L]).

Strategy:
  Phase 1 (projection): shard the 65536 (b,l,hh,ww) positions across the 8
  cores; each core computes all 256 output channels for its 8192 positions
  (three 256x256 channel matmuls). Weights are tiny and replicated.
  Phase 2 (attention): shard heads across cores (core h <-> head h, both
  batches). Host reshuffles phase-1 output into per-head transposed layouts:
  QT/KT [512=(d,hh,ww), L] and V [L, 512]. On-chip: S^T = K^T-tiles @ Q-tiles
  (PSUM, f32r matmuls), exp on ScalarE, row sums over keys via ones-matmul,
  reciprocal, context = P^T-tiles @ V-tiles scaled by 1/rowsum. attn is
  written key-major (transposed) and untransposed on host.
"""

import os
import numpy as np

import concourse.bass as bass
import concourse.mybir as mybir
from concourse import tile
from concourse.bass_utils import run_bass_kernel_spmd
from concourse.masks import make_identity

F32 = mybir.dt.float32
F32R = mybir.dt.float32r

NCORES = 8
B, L, C, H, W = 2, 2048, 256, 4, 4
NH, DH = 8, C // 8
DHW = DH * H * W          # 512 contraction/feature size per head
POS = B * L * H * W       # 65536 projection positions
PPC = POS // NCORES       # 8192 positions per core
SCALE = 1.0 / np.sqrt(np.float32(DH))

# exec times (ns) of the last run, per phase, when tracing is enabled
LAST_EXEC_NS = {}
_TRACE = bool(int(os.environ.get("BASS_ATTN_TRACE", "0")))


def _install_ntff_hook():
    import sys, types
    try:
        import antenv.axon_hooks  # noqa: F401
        return
    except ImportError:
        pass
    try:
        import antenv
        mod = types.ModuleType("antenv.axon_hooks")
        mod._hook = None
        mod.set_axon_ntff_profile_hook = lambda h: setattr(mod, "_hook", h)
        mod.get_axon_ntff_profile_hook = lambda: mod._hook
        sys.modules["antenv.axon_hooks"] = mod
        antenv.axon_hooks = mod
        if "/root/.axon_site" not in sys.path:
            sys.path.insert(0, "/root/.axon_site")
        from trn_agent_boot.trn_boot import _ntff_profile_via_ctypes
        mod.set_axon_ntff_profile_hook(
            _ntff_profile_via_ctypes("/opt/axon/libaxon_pjrt.so"))
    except Exception:
        pass


def _split_waits(nc, max_waits=1):
    """This walrus build rejects >1 sync wait per instruction; hoist excess
    waits onto preceding same-engine NOPs (semantics preserved: engine
    streams execute in order)."""
    for f in nc.m.functions:
        for b in f.blocks:
            new_list = []
            changed = False
            for inst in b.instructions:
                si = inst.sync_info
                waits = list(si.on_wait) if si else []
                if si and len(waits) > max_waits:
                    changed = True
                    excess, keep = waits[:-max_waits], waits[-max_waits:]
                    for i in range(0, len(excess), max_waits):
                        new_list.append(mybir.InstNoOp(
                            name=nc.get_next_instruction_name(),
                            ins=[], outs=[], engine=inst.engine,
                            sync_info=mybir.SyncInfo(
                                on_wait=excess[i:i + max_waits], on_update=[]),
                        ))
                    inst.sync_info = mybir.SyncInfo(
                        on_wait=keep, on_update=list(si.on_update))
                new_list.append(inst)
            if changed:
                b.instructions = new_list


def _build_phase1():
    """Per core: Y[t] [256, PPC] = W[t]^T-laid @ X[t] [256, PPC] for t in
    q,k,v.  X is channel-major (positions in free dim)."""
    nc = bass.Bass("TRN2", target_bir_lowering=False, debug=False)
    xs, ws, ys = [], [], []
    for t in ("q", "k", "v"):
        xs.append(nc.dram_tensor(f"X{t}", [C, PPC], F32R, kind="ExternalInput").ap())
        ws.append(nc.dram_tensor(f"W{t}", [C, C], F32R, kind="ExternalInput").ap())
        ys.append(nc.dram_tensor(f"Y{t}", [C, PPC], F32, kind="ExternalOutput").ap())

    NCC = C // 128            # 2 contraction chunks
    NOC = C // 128            # 2 output-channel chunks
    NPT = PPC // 512          # 16 position tiles
    NQ = 4                    # output DMA quarters
    with tile.TileContext(nc) as tc:
        with (
            tc.tile_pool(name="wp", bufs=1) as wp,
            tc.tile_pool(name="xp", bufs=13) as xp,
            tc.tile_pool(name="yp", bufs=2) as yp,
            tc.tile_pool(name="ps", bufs=4, space="PSUM") as ps,
        ):
            wt = []
            for t in range(3):
                w = wp.tile([128, NCC, C], F32R, tag=f"w{t}")
                nc.gpsimd.dma_start(
                    w[:], ws[t].rearrange("(cc p) o -> p cc o", p=128))
                wt.append(w)
            NXQ = 4   # X load quarters
            for t in range(3):
                xcc = [[None] * NXQ for _ in range(NCC)]
                for xq in range(NXQ):
                    for cc in range(NCC):
                        x = xp.tile([128, PPC // NXQ], F32R, tag="x")
                        eng = nc.sync if cc == 0 else nc.scalar
                        eng.dma_start(
                            x[:], xs[t][cc * 128:(cc + 1) * 128,
                                        xq * (PPC // NXQ):(xq + 1) * (PPC // NXQ)])
                        xcc[cc][xq] = x
                for oc in range(NOC):
                    y = yp.tile([128, PPC], F32, tag="y")
                    for q in range(NQ):
                        for p in range(q * NPT // NQ, (q + 1) * NPT // NQ):
                            s = ps.tile([128, 512], F32, tag="s")
                            for cc in range(NCC):
                                xt_ = xcc[cc][p // 4]
                                nc.tensor.matmul(
                                    s[:],
                                    wt[t][:, cc, oc * 128:(oc + 1) * 128],
                                    xt_[:, (p % 4) * 512:(p % 4 + 1) * 512],
                                    start=(cc == 0), stop=(cc == NCC - 1))
                            if p % 2 == 0:
                                nc.vector.tensor_copy(
                                    y[:, p * 512:(p + 1) * 512], s[:])
                            else:
                                nc.scalar.copy(
                                    y[:, p * 512:(p + 1) * 512], s[:])
                        q0 = q * (PPC // NQ)
                        (nc.gpsimd if q % 2 == 0 else nc.sync).dma_start(
                            ys[t][oc * 128:(oc + 1) * 128, q0:q0 + PPC // NQ],
                            y[:, q0:q0 + PPC // NQ])
    _split_waits(nc)
    return nc


def _build_phase2():
    """Per core (= one head, both batches): S^T/softmax/PV.
    Inputs: QT, KT [2, 512, L] (query pre-scaled), V [2, L, 512].
    Outputs: CO [2, L, 512] context, AT [2, L, L] attn transposed [j, i]."""
    nc = bass.Bass("TRN2", target_bir_lowering=False, debug=False)
    QT = nc.dram_tensor("QT", [B, DHW, L], F32R, kind="ExternalInput").ap()
    KT = nc.dram_tensor("KT", [B, DHW, L], F32R, kind="ExternalInput").ap()
    V = nc.dram_tensor("V", [B, L, DHW], F32R, kind="ExternalInput").ap()
    ONES = nc.dram_tensor("ONES", [128, 1], F32R, kind="ExternalInput").ap()
    CO = nc.dram_tensor("CO", [B, L, DHW], F32, kind="ExternalOutput").ap()
    AT = nc.dram_tensor("AT", [B, L, L], F32, kind="ExternalOutput").ap()

    ND = DHW // 128   # 4 contraction chunks (d,hh,ww)
    NJ = L // 128     # 16 key chunks
    NI = L // 512     # 4 query blocks
    with tile.TileContext(nc) as tc:
        with (
            tc.tile_pool(name="cst", bufs=1) as cst,
            tc.tile_pool(name="kqv", bufs=17) as kqv,
            tc.tile_pool(name="vp", bufs=4) as vp,
            tc.tile_pool(name="ptp", bufs=2) as ptp,
            tc.tile_pool(name="ev", bufs=2) as ev,
            tc.tile_pool(name="ptnp", bufs=2) as ptnp,
            tc.tile_pool(name="sm", bufs=1) as sm,
            tc.tile_pool(name="psS", bufs=2, space="PSUM") as psS,
            tc.tile_pool(name="psC", bufs=2, space="PSUM") as psC,
            tc.tile_pool(name="psR", bufs=1, space="PSUM") as psR,
        ):
            ones = cst.tile([128, 1], F32R)
            nc.gpsimd.dma_start(ones[:], ONES[:])
            ones32 = cst.tile([128, 1], F32)
            nc.vector.memset(ones32[:], 1.0)
            ones_row = cst.tile([1, 128], F32)
            nc.vector.memset(ones_row[:], 1.0)
            ident = cst.tile([128, 128], F32)
            make_identity(nc, ident[:])

            for fb in range(B):
                kt = [[None] * 4 for _ in range(ND)]
                qt = [[None] * 4 for _ in range(ND)]
                for lq in range(4):
                    for dc in range(ND):
                        k_ = kqv.tile([128, 512], F32R, tag="kt")
                        nc.sync.dma_start(
                            k_[:], KT[fb, dc * 128:(dc + 1) * 128,
                                      lq * 512:(lq + 1) * 512])
                        kt[dc][lq] = k_
                        q_ = kqv.tile([128, 512], F32R, tag="qt")
                        nc.gpsimd.dma_start(
                            q_[:], QT[fb, dc * 128:(dc + 1) * 128,
                                      lq * 512:(lq + 1) * 512])
                        qt[dc][lq] = q_
                v = []
                for vq in range(4):
                    v_ = vp.tile([128, 4, DHW], F32R, tag="v")
                    nc.sync.dma_start(
                        v_[:], V[fb, vq * 512:(vq + 1) * 512, :].rearrange(
                            "(g p) c -> p g c", p=128))
                    v.append(v_)

                for ib in range(NI):
                    pt = ptp.tile([128, NJ, 512], F32R, tag="pt")
                    sums = psR.tile([1, 512], F32, tag="sums")
                    for jc in range(NJ):
                        s = psS.tile([128, 512], F32, tag="s")
                        for dc in range(ND):
                            nc.tensor.matmul(
                                s[:],
                                kt[dc][jc // 4][:, (jc % 4) * 128:
                                                (jc % 4 + 1) * 128],
                                qt[dc][ib][:],
                                start=(dc == 0), stop=(dc == ND - 1))
                        nc.scalar.activation(
                            pt[:, jc, :], s[:],
                            mybir.ActivationFunctionType.Exp)
                        nc.tensor.matmul(
                            sums[:], ones[:], pt[:, jc, :],
                            start=(jc == 0), stop=(jc == NJ - 1))
                    sums_sb = sm.tile([1, 512], F32, tag="sums_sb")
                    nc.scalar.copy(sums_sb[:], sums[:])
                    rec = sm.tile([128, 4], F32, tag="rec")
                    for ic in range(4):
                        tcol = psR.tile([128, 1], F32, tag="tcol")
                        nc.tensor.matmul(
                            tcol[:], sums_sb[:, ic * 128:(ic + 1) * 128],
                            ones32[:1, :1], start=True, stop=True)
                        nc.vector.tensor_copy(rec[:, ic:ic + 1], tcol[:])
                    nc.vector.reciprocal(rec[:], rec[:])

                    # broadcast 1/rowsum along key partitions:
                    # rec cols [128i,1] -> rows [1,128i] -> recB [128j, 512i]
                    rrow = sm.tile([1, 512], F32, tag="rrow")
                    for ic in range(4):
                        rT = psR.tile([1, 128], F32, tag="rT")
                        nc.tensor.transpose(
                            rT[:], rec[:, ic:ic + 1], ident[:])
                        nc.vector.tensor_copy(
                            rrow[:, ic * 128:(ic + 1) * 128], rT[:])
                    recB = psR.tile([128, 512], F32, tag="recB")
                    nc.tensor.matmul(recB[:], ones_row[:], rrow[:],
                                     start=True, stop=True)
                    recB_sb = sm.tile([128, 512], F32, tag="recB_sb")
                    nc.vector.tensor_copy(recB_sb[:], recB[:])
                    for g in range(4):
                        ptn = ptnp.tile([128, 4, 512], F32, tag="ptn")
                        for j2 in range(4):
                            nc.vector.tensor_mul(
                                ptn[:, j2, :],
                                pt[:, g * 4 + j2, :].bitcast(F32),
                                recB_sb[:])
                        nc.sync.dma_start(
                            AT[fb, g * 512:(g + 1) * 512,
                               ib * 512:(ib + 1) * 512].rearrange(
                                "(g2 p) i -> p g2 i", p=128),
                            ptn[:])

                    osb = ev.tile([128, 4, DHW], F32, tag="osb")
                    for ic in range(4):
                        o = psC.tile([128, DHW], F32, tag="o")
                        for jc in range(NJ):
                            nc.tensor.matmul(
                                o[:],
                                pt[:, jc, ic * 128:(ic + 1) * 128],
                                v[jc // 4][:, jc % 4, :],
                                start=(jc == 0), stop=(jc == NJ - 1))
                        nc.vector.tensor_scalar_mul(
                            osb[:, ic, :], o[:], rec[:, ic:ic + 1])
                    nc.scalar.dma_start(
                        CO[fb, ib * 512:(ib + 1) * 512, :].rearrange(
                            "(g p) c -> p g c", p=128),
                        osb[:])
    _split_waits(nc)
    return nc


_programs = {}


def _get_programs():
    if not _programs:
        _programs["p1"] = _build_phase1()
        _programs["p2"] = _build_phase2()
    return _programs


def kernel(query, key, value, Wq, Wk, Wv):
    query = np.asarray(query, np.float32)
    key = np.asarray(key, np.float32)
    value = np.asarray(value, np.float32)
    Wq = np.asarray(Wq, np.float32)
    Wk = np.asarray(Wk, np.float32)
    Wv = np.asarray(Wv, np.float32)
    if _TRACE:
        _install_ntff_hook()
    progs = _get_programs()
    cores = list(range(NCORES))

    # ---- phase 1: channel-major inputs, shard positions across cores ----
    xq = np.ascontiguousarray(
        query.transpose(2, 0, 1, 3, 4).reshape(C, POS))
    xk = np.ascontiguousarray(
        key.transpose(2, 0, 1, 3, 4).reshape(C, POS))
    xv = np.ascontiguousarray(
        value.transpose(2, 0, 1, 3, 4).reshape(C, POS))
    wqT = np.ascontiguousarray((Wq * SCALE).T)   # [c, o], scale folded
    wkT = np.ascontiguousarray(Wk.T)
    wvT = np.ascontiguousarray(Wv.T)

    in1 = [{
        "Xq": np.ascontiguousarray(xq[:, c * PPC:(c + 1) * PPC]),
        "Xk": np.ascontiguousarray(xk[:, c * PPC:(c + 1) * PPC]),
        "Xv": np.ascontiguousarray(xv[:, c * PPC:(c + 1) * PPC]),
        "Wq": wqT, "Wk": wkT, "Wv": wvT,
    } for c in cores]
    r1 = run_bass_kernel_spmd(progs["p1"], in1, cores, trace=_TRACE)
    LAST_EXEC_NS["phase1"] = r1.exec_time_ns

    yq = np.concatenate([r1.results[c]["Yq"] for c in cores], axis=1)
    yk = np.concatenate([r1.results[c]["Yk"] for c in cores], axis=1)
    yv = np.concatenate([r1.results[c]["Yv"] for c in cores], axis=1)

    # ---- phase 2: per-head transposed layouts ----
    # y* [256, POS] -> [nh, dh, B, L, H, W]
    yq = yq.reshape(NH, DH, B, L, H, W)
    yk = yk.reshape(NH, DH, B, L, H, W)
    yv = yv.reshape(NH, DH, B, L, H, W)
    ones = np.ones((128, 1), np.float32)
    in2 = []
    for h in cores:
        # QT/KT: [B, (d,hh,ww)=512, L]
        qt = np.ascontiguousarray(
            yq[h].transpose(1, 0, 3, 4, 2).reshape(B, DHW, L))
        kt = np.ascontiguousarray(
            yk[h].transpose(1, 0, 3, 4, 2).reshape(B, DHW, L))
        # V: [B, L, (d,hh,ww)]
        v = np.ascontiguousarray(
            yv[h].transpose(1, 2, 0, 3, 4).reshape(B, L, DHW))
        in2.append({"QT": qt, "KT": kt, "V": v, "ONES": ones})
    r2 = run_bass_kernel_spmd(progs["p2"], in2, cores, trace=_TRACE)
    LAST_EXEC_NS["phase2"] = r2.exec_time_ns

    # ---- unshard ----
    context = np.empty((B, L, C, H, W), np.float32)
    attn = np.empty((NH * B, L, L), np.float32)
    for h in cores:
        co = r2.results[h]["CO"].reshape(B, L, DH, H, W)
        at = r2.results[h]["AT"]
        for b in range(B):
            context[b, :, h * DH:(h + 1) * DH, :, :] = co[b]
            attn[h * B + b] = at[b].T
    return context, attn


# revision 15
# speedup vs baseline: 1.0631x; 1.0631x over previous
"""Multi-head attention (conv1x1 projections) on 8 Trainium2 NeuronCores.

Reference computation (B=2, L=2048, C=256, H=W=4, 8 heads x d_head=32):
    q = einsum('blchw,oc->blohw', query, Wq)   (same for k, v)
    per flattened batch fb = head*B + b:
      score = q_fb @ k_fb^T / sqrt(32)   -> softmax over keys -> attn
      context = attn @ v_fb
Returns (context [B,L,C,H,W], attn [16,L,L]).

Strategy:
  Phase 1 (projection): shard the 65536 (b,l,hh,ww) positions across the 8
  cores; each core computes all 256 output channels for its 8192 positions
  (three 256x256 channel matmuls). Weights are tiny and replicated.
  Phase 2 (attention): shard heads across cores (core h <-> head h, both
  batches). Host reshuffles phase-1 output into per-head transposed layouts:
  QT/KT [512=(d,hh,ww), L] and V [L, 512]. On-chip: S^T = K^T-tiles @ Q-tiles
  (PSUM, f32r matmuls), exp on ScalarE, row sums over keys via ones-matmul,
  reciprocal, context = P^T-tiles @ V-tiles scaled by 1/rowsum. attn is
  written key-major (transposed) and untransposed on host.
"""

import os
import numpy as np

import concourse.bass as bass
import concourse.mybir as mybir
from concourse import tile
from concourse.bass_utils import run_bass_kernel_spmd
from concourse.masks import make_identity

F32 = mybir.dt.float32
F32R = mybir.dt.float32r

NCORES = 8
B, L, C, H, W = 2, 2048, 256, 4, 4
NH, DH = 8, C // 8
DHW = DH * H * W          # 512 contraction/feature size per head
POS = B * L * H * W       # 65536 projection positions
PPC = POS // NCORES       # 8192 positions per core
SCALE = 1.0 / np.sqrt(np.float32(DH))

# exec times (ns) of the last run, per phase, when tracing is enabled
LAST_EXEC_NS = {}
_TRACE = bool(int(os.environ.get("BASS_ATTN_TRACE", "0")))


def _install_ntff_hook():
    import sys, types
    try:
        import antenv.axon_hooks  # noqa: F401
        return
    except ImportError:
        pass
    try:
        import antenv
        mod = types.ModuleType("antenv.axon_hooks")
        mod._hook = None
        mod.set_axon_ntff_profile_hook = lambda h: setattr(mod, "_hook", h)
        mod.get_axon_ntff_profile_hook = lambda: mod._hook
        sys.modules["antenv.axon_hooks"] = mod
        antenv.axon_hooks = mod
        if "/root/.axon_site" not in sys.path:
            sys.path.insert(0, "/root/.axon_site")
        from trn_agent_boot.trn_boot import _ntff_profile_via_ctypes
        mod.set_axon_ntff_profile_hook(
            _ntff_profile_via_ctypes("/opt/axon/libaxon_pjrt.so"))
    except Exception:
        pass


def _split_waits(nc, max_waits=1):
    """This walrus build rejects >1 sync wait per instruction; hoist excess
    waits onto preceding same-engine NOPs (semantics preserved: engine
    streams execute in order)."""
    for f in nc.m.functions:
        for b in f.blocks:
            new_list = []
            changed = False
            for inst in b.instructions:
                si = inst.sync_info
                waits = list(si.on_wait) if si else []
                if si and len(waits) > max_waits:
                    changed = True
                    excess, keep = waits[:-max_waits], waits[-max_waits:]
                    for i in range(0, len(excess), max_waits):
                        new_list.append(mybir.InstNoOp(
                            name=nc.get_next_instruction_name(),
                            ins=[], outs=[], engine=inst.engine,
                            sync_info=mybir.SyncInfo(
                                on_wait=excess[i:i + max_waits], on_update=[]),
                        ))
                    inst.sync_info = mybir.SyncInfo(
                        on_wait=keep, on_update=list(si.on_update))
                new_list.append(inst)
            if changed:
                b.instructions = new_list


def _build_phase1():
    """Per core: Y[t] [256, PPC] = W[t]^T-laid @ X[t] [256, PPC] for t in
    q,k,v.  X is channel-major (positions in free dim)."""
    nc = bass.Bass("TRN2", target_bir_lowering=False, debug=False)
    xs, ws, ys = [], [], []
    for t in ("q", "k", "v"):
        xs.append(nc.dram_tensor(f"X{t}", [C, PPC], F32R, kind="ExternalInput").ap())
        ws.append(nc.dram_tensor(f"W{t}", [C, C], F32R, kind="ExternalInput").ap())
        ys.append(nc.dram_tensor(f"Y{t}", [C, PPC], F32, kind="ExternalOutput").ap())

    NCC = C // 128            # 2 contraction chunks
    NOC = C // 128            # 2 output-channel chunks
    NPT = PPC // 512          # 16 position tiles
    NQ = 4                    # output DMA quarters
    with tile.TileContext(nc) as tc:
        with (
            tc.tile_pool(name="wp", bufs=1) as wp,
            tc.tile_pool(name="xp", bufs=13) as xp,
            tc.tile_pool(name="yp", bufs=2) as yp,
            tc.tile_pool(name="ps", bufs=4, space="PSUM") as ps,
        ):
            wt = []
            for t in range(3):
                w = wp.tile([128, NCC, C], F32R, tag=f"w{t}")
                nc.gpsimd.dma_start(
                    w[:], ws[t].rearrange("(cc p) o -> p cc o", p=128))
                wt.append(w)
            NXQ = 4   # X load quarters
            for t in range(3):
                xcc = [[None] * NXQ for _ in range(NCC)]
                for xq in range(NXQ):
                    for cc in range(NCC):
                        x = xp.tile([128, PPC // NXQ], F32R, tag="x")
                        eng = nc.sync if cc == 0 else nc.scalar
                        eng.dma_start(
                            x[:], xs[t][cc * 128:(cc + 1) * 128,
                                        xq * (PPC // NXQ):(xq + 1) * (PPC // NXQ)])
                        xcc[cc][xq] = x
                for oc in range(NOC):
                    y = yp.tile([128, PPC], F32, tag="y")
                    for q in range(NQ):
                        for p in range(q * NPT // NQ, (q + 1) * NPT // NQ):
                            s = ps.tile([128, 512], F32, tag="s")
                            for cc in range(NCC):
                                xt_ = xcc[cc][p // 4]
                                nc.tensor.matmul(
                                    s[:],
                                    wt[t][:, cc, oc * 128:(oc + 1) * 128],
                                    xt_[:, (p % 4) * 512:(p % 4 + 1) * 512],
                                    start=(cc == 0), stop=(cc == NCC - 1))
                            nc.vector.tensor_copy(
                                y[:, p * 512:(p + 1) * 512], s[:])
                        q0 = q * (PPC // NQ)
                        (nc.gpsimd if q % 2 == 0 else nc.sync).dma_start(
                            ys[t][oc * 128:(oc + 1) * 128, q0:q0 + PPC // NQ],
                            y[:, q0:q0 + PPC // NQ])
    _split_waits(nc)
    return nc


def _build_phase2():
    """Per core (= one head, both batches): S^T/softmax/PV.
    Inputs: QT, KT [2, 512, L] (query pre-scaled), V [2, L, 512].
    Outputs: CO [2, L, 512] context, AT [2, L, L] attn transposed [j, i]."""
    nc = bass.Bass("TRN2", target_bir_lowering=False, debug=False)
    QT = nc.dram_tensor("QT", [B, DHW, L], F32R, kind="ExternalInput").ap()
    KT = nc.dram_tensor("KT", [B, DHW, L], F32R, kind="ExternalInput").ap()
    V = nc.dram_tensor("V", [B, L, DHW], F32R, kind="ExternalInput").ap()
    ONES = nc.dram_tensor("ONES", [128, 1], F32R, kind="ExternalInput").ap()
    CO = nc.dram_tensor("CO", [B, L, DHW], F32, kind="ExternalOutput").ap()
    AT = nc.dram_tensor("AT", [B, L, L], F32, kind="ExternalOutput").ap()

    ND = DHW // 128   # 4 contraction chunks (d,hh,ww)
    NJ = L // 128     # 16 key chunks
    NI = L // 512     # 4 query blocks
    with tile.TileContext(nc) as tc:
        with (
            tc.tile_pool(name="cst", bufs=1) as cst,
            tc.tile_pool(name="kqv", bufs=17) as kqv,
            tc.tile_pool(name="vp", bufs=4) as vp,
            tc.tile_pool(name="ptp", bufs=2) as ptp,
            tc.tile_pool(name="ev", bufs=2) as ev,
            tc.tile_pool(name="ptnp", bufs=2) as ptnp,
            tc.tile_pool(name="sm", bufs=1) as sm,
            tc.tile_pool(name="psS", bufs=2, space="PSUM") as psS,
            tc.tile_pool(name="psC", bufs=2, space="PSUM") as psC,
            tc.tile_pool(name="psR", bufs=1, space="PSUM") as psR,
        ):
            ones = cst.tile([128, 1], F32R)
            nc.gpsimd.dma_start(ones[:], ONES[:])
            ones32 = cst.tile([128, 1], F32)
            nc.vector.memset(ones32[:], 1.0)
            ones_row = cst.tile([1, 128], F32)
            nc.vector.memset(ones_row[:], 1.0)
            ident = cst.tile([128, 128], F32)
            make_identity(nc, ident[:])

            for fb in range(B):
                kt = [[None] * 4 for _ in range(ND)]
                qt = [[None] * 4 for _ in range(ND)]
                for lq in range(4):
                    for dc in range(ND):
                        k_ = kqv.tile([128, 512], F32R, tag="kt")
                        nc.sync.dma_start(
                            k_[:], KT[fb, dc * 128:(dc + 1) * 128,
                                      lq * 512:(lq + 1) * 512])
                        kt[dc][lq] = k_
                        q_ = kqv.tile([128, 512], F32R, tag="qt")
                        nc.gpsimd.dma_start(
                            q_[:], QT[fb, dc * 128:(dc + 1) * 128,
                                      lq * 512:(lq + 1) * 512])
                        qt[dc][lq] = q_
                v = []
                for vq in range(4):
                    v_ = vp.tile([128, 4, DHW], F32R, tag="v")
                    nc.sync.dma_start(
                        v_[:], V[fb, vq * 512:(vq + 1) * 512, :].rearrange(
                            "(g p) c -> p g c", p=128))
                    v.append(v_)

                for ib in range(NI):
                    pt = ptp.tile([128, NJ, 512], F32R, tag="pt")
                    sums = psR.tile([1, 512], F32, tag="sums")
                    for jc in range(NJ):
                        s = psS.tile([128, 512], F32, tag="s")
                        for dc in range(ND):
                            nc.tensor.matmul(
                                s[:],
                                kt[dc][jc // 4][:, (jc % 4) * 128:
                                                (jc % 4 + 1) * 128],
                                qt[dc][ib][:],
                                start=(dc == 0), stop=(dc == ND - 1))
                        nc.scalar.activation(
                            pt[:, jc, :], s[:],
                            mybir.ActivationFunctionType.Exp)
                        nc.tensor.matmul(
                            sums[:], ones[:], pt[:, jc, :],
                            start=(jc == 0), stop=(jc == NJ - 1))
                    sums_sb = sm.tile([1, 512], F32, tag="sums_sb")
                    nc.scalar.copy(sums_sb[:], sums[:])
                    rec = sm.tile([128, 4], F32, tag="rec")
                    for ic in range(4):
                        tcol = psR.tile([128, 1], F32, tag="tcol")
                        nc.tensor.matmul(
                            tcol[:], sums_sb[:, ic * 128:(ic + 1) * 128],
                            ones32[:1, :1], start=True, stop=True)
                        nc.vector.tensor_copy(rec[:, ic:ic + 1], tcol[:])
                    nc.vector.reciprocal(rec[:], rec[:])

                    # broadcast 1/rowsum along key partitions:
                    # rec cols [128i,1] -> rows [1,128i] -> recB [128j, 512i]
                    rrow = sm.tile([1, 512], F32, tag="rrow")
                    for ic in range(4):
                        rT = psR.tile([1, 128], F32, tag="rT")
                        nc.tensor.transpose(
                            rT[:], rec[:, ic:ic + 1], ident[:])
                        nc.vector.tensor_copy(
                            rrow[:, ic * 128:(ic + 1) * 128], rT[:])
                    recB = psR.tile([128, 512], F32, tag="recB")
                    nc.tensor.matmul(recB[:], ones_row[:], rrow[:],
                                     start=True, stop=True)
                    recB_sb = sm.tile([128, 512], F32, tag="recB_sb")
                    nc.vector.tensor_copy(recB_sb[:], recB[:])
                    for g in range(4):
                        ptn = ptnp.tile([128, 4, 512], F32, tag="ptn")
                        for j2 in range(4):
                            nc.vector.tensor_mul(
                                ptn[:, j2, :],
                                pt[:, g * 4 + j2, :].bitcast(F32),
                                recB_sb[:])
                        nc.sync.dma_start(
                            AT[fb, g * 512:(g + 1) * 512,
                               ib * 512:(ib + 1) * 512].rearrange(
                                "(g2 p) i -> p g2 i", p=128),
                            ptn[:])

                    osb = ev.tile([128, 4, DHW], F32, tag="osb")
                    for ic in range(4):
                        o = psC.tile([128, DHW], F32, tag="o")
                        for jc in range(NJ):
                            nc.tensor.matmul(
                                o[:],
                                pt[:, jc, ic * 128:(ic + 1) * 128],
                                v[jc // 4][:, jc % 4, :],
                                start=(jc == 0), stop=(jc == NJ - 1))
                        nc.vector.tensor_scalar_mul(
                            osb[:, ic, :], o[:], rec[:, ic:ic + 1])
                    nc.scalar.dma_start(
                        CO[fb, ib * 512:(ib + 1) * 512, :].rearrange(
                            "(g p) c -> p g c", p=128),
                        osb[:])
    _split_waits(nc)
    return nc


_programs = {}


def _get_programs():
    if not _programs:
        _programs["p1"] = _build_phase1()
        _programs["p2"] = _build_phase2()
    return _programs


def kernel(query, key, value, Wq, Wk, Wv):
    query = np.asarray(query, np.float32)
    key = np.asarray(key, np.float32)
    value = np.asarray(value, np.float32)
    Wq = np.asarray(Wq, np.float32)
    Wk = np.asarray(Wk, np.float32)
    Wv = np.asarray(Wv, np.float32)
    if _TRACE:
        _install_ntff_hook()
    progs = _get_programs()
    cores = list(range(NCORES))

    # ---- phase 1: channel-major inputs, shard positions across cores ----
    xq = np.ascontiguousarray(
        query.transpose(2, 0, 1, 3, 4).reshape(C, POS))
    xk = np.ascontiguousarray(
        key.transpose(2, 0, 1, 3, 4).reshape(C, POS))
    xv = np.ascontiguousarray(
        value.transpose(2, 0, 1, 3, 4).reshape(C, POS))
    wqT = np.ascontiguousarray((Wq * SCALE).T)   # [c, o], scale folded
    wkT = np.ascontiguousarray(Wk.T)
    wvT = np.ascontiguousarray(Wv.T)

    in1 = [{
        "Xq": np.ascontiguousarray(xq[:, c * PPC:(c + 1) * PPC]),
        "Xk": np.ascontiguousarray(xk[:, c * PPC:(c + 1) * PPC]),
        "Xv": np.ascontiguousarray(xv[:, c * PPC:(c + 1) * PPC]),
        "Wq": wqT, "Wk": wkT, "Wv": wvT,
    } for c in cores]
    r1 = run_bass_kernel_spmd(progs["p1"], in1, cores, trace=_TRACE)
    LAST_EXEC_NS["phase1"] = r1.exec_time_ns

    yq = np.concatenate([r1.results[c]["Yq"] for c in cores], axis=1)
    yk = np.concatenate([r1.results[c]["Yk"] for c in cores], axis=1)
    yv = np.concatenate([r1.results[c]["Yv"] for c in cores], axis=1)

    # ---- phase 2: per-head transposed layouts ----
    # y* [256, POS] -> [nh, dh, B, L, H, W]
    yq = yq.reshape(NH, DH, B, L, H, W)
    yk = yk.reshape(NH, DH, B, L, H, W)
    yv = yv.reshape(NH, DH, B, L, H, W)
    ones = np.ones((128, 1), np.float32)
    in2 = []
    for h in cores:
        # QT/KT: [B, (d,hh,ww)=512, L]
        qt = np.ascontiguousarray(
            yq[h].transpose(1, 0, 3, 4, 2).reshape(B, DHW, L))
        kt = np.ascontiguousarray(
            yk[h].transpose(1, 0, 3, 4, 2).reshape(B, DHW, L))
        # V: [B, L, (d,hh,ww)]
        v = np.ascontiguousarray(
            yv[h].transpose(1, 2, 0, 3, 4).reshape(B, L, DHW))
        in2.append({"QT": qt, "KT": kt, "V": v, "ONES": ones})
    r2 = run_bass_kernel_spmd(progs["p2"], in2, cores, trace=_TRACE)
    LAST_EXEC_NS["phase2"] = r2.exec_time_ns

    # ---- unshard ----
    context = np.empty((B, L, C, H, W), np.float32)
    attn = np.empty((NH * B, L, L), np.float32)
    for h in cores:
        co = r2.results[h]["CO"].reshape(B, L, DH, H, W)
        at = r2.results[h]["AT"]
        for b in range(B):
            context[b, :, h * DH:(h + 1) * DH, :, :] = co[b]
            attn[h * B + b] = at[b].T
    return context, attn
